# revision 6
# baseline (speedup 1.0000x reference)
"""Trainium2 kernel for a fuzzy-logic ConjunctionLayer forward pass.

Computes  out = 1[ (1 - x) @ 1[W > 0.5] <= 0 ]  for
x: [8192, 4096] f32, W: [4096, 2048] f32 -> out: [8192, 2048] f32.

Sharding: data-parallel over the batch dim across 8 NeuronCores
(x shard [1024, 4096] per core, W replicated), outputs concatenated.

Math: with x in [0, 1], every term (1-x)*Wb is >= 0, so
  res[m,n] <= 0  <=>  res[m,n] == 0  <=>  no k has (x[m,k] < 1 AND W[k,n] > .5).
The output depends only on the support pattern, so both operands are
binarized on device:
  s  = 1[x < 1],  Wb = 1[W > .5]   (both {0,1}, exact in fp8e4)
  acc = s^T.T @ Wb                 (f32 PSUM accumulation - exact integers)
  out = 1[acc <= 0]
fp8 enables the PE DoubleRow perf mode (2 fp8 weights per cell -> 2x
MACs/cycle, contraction 256 per matmul).

The kernel is DMA-bound (56 MB/core), so the schedule maximizes HBM
bandwidth: every transfer is >= 0.5 MB with >= 4 KB contiguous
per-partition rows (the host pre-permutes x and W into k-pair-major
layouts to make that possible), transfers alternate across the two HWDGE
rings (Sync + Scalar issue queues), and the 8 batch-chunk accumulation
chains (one PSUM bank each) consume every k-pair the moment it lands.
W streams n-block-major so each 8 MB block is reused by all 8 chains
while the next block prefetches; output stores are split between the
GPSIMD SWDGE queue and the rings.
"""

import numpy as np

import concourse.bass as bass
import concourse.mybir as mybir
import concourse.tile as tile
from concourse import bacc
from concourse.bass_utils import run_bass_kernel_spmd

BATCH, IN_DIM, N_RULES = 8192, 4096, 2048
N_CORES = 8
M_LOCAL = BATCH // N_CORES  # 1024 batch rows per core

P = 128            # SBUF partitions / matmul tile edge
NB_W = 512         # n-block width (= one f32 PSUM bank)
NB = N_RULES // NB_W        # 4 n-blocks
KT = IN_DIM // P            # 32 k-tiles
KP = KT // 2                # 16 k-pairs (DoubleRow consumes 2 per matmul)
MT = M_LOCAL // P           # 8 batch chunks per core

F32 = mybir.dt.float32
FP8 = mybir.dt.float8e4
ALU = mybir.AluOpType
DR = mybir.MatmulPerfMode.DoubleRow


def _body(tc: tile.TileContext, out: bass.AP, xp: bass.AP, wp: bass.AP):
    nc = tc.nc
    rings = (nc.sync, nc.scalar)  # the two HWDGE issue queues
    with (
        tc.tile_pool(name="sb", bufs=1) as sb,
        tc.tile_pool(name="ps", bufs=1, space="PSUM") as ps,
    ):
        # Resident binarized operands (2D tiles; matmul slices them as
        # [128, 2, .] k-pair APs via rearrange).
        s2 = [sb.tile([P, 2 * M_LOCAL], FP8, tag=f"s{kk}", bufs=1,
                      name=f"s{kk}") for kk in range(KP)]
        wb2 = [[sb.tile([P, 2 * NB_W], FP8, tag=f"wb{nb}_{kk}", bufs=1,
                        name=f"wb{nb}_{kk}") for kk in range(KP)]
               for nb in range(NB)]
        # nb0|nb1 output halves paired per batch chunk for 4 KB-row stores
        o2 = [sb.tile([P, 2 * NB_W], F32, tag=f"o2_{m}", bufs=1,
                      name=f"o2_{m}") for m in range(MT)]

        def load_x_pair(kk, ring):
            xf = sb.tile([P, 2 * M_LOCAL], F32, tag="xf", bufs=2,
                         name=f"xf{kk}")
            ring.dma_start(xf[:], xp[kk])
            nc.vector.tensor_scalar(s2[kk][:], xf[:], 1.0, None, ALU.is_lt)

        def load_w_pair(nb, kk, ring):
            wf = sb.tile([P, 2 * NB_W], F32, tag="wf", bufs=4,
                         name=f"wf{nb}_{kk}")
            ring.dma_start(wf[:], wp[kk * NB + nb])
            nc.vector.tensor_scalar(wb2[nb][kk][:], wf[:], 0.5, None,
                                    ALU.is_gt)

        accs = {}

        def mm_step(nb, kk):
            """All 8 batch chains consume k-pair kk of n-block nb."""
            rhs = wb2[nb][kk][:].rearrange("p (two n) -> p two n", two=2)
            lhsT = s2[kk][:].rearrange("p (two m) -> p two m", two=2)
            for m in range(MT):
                if kk == 0:
                    accs[m] = ps.tile([P, NB_W], F32, tag=f"acc{m}", bufs=1,
                                      name=f"acc{nb}_{m}")
                nc.tensor.matmul(
                    accs[m][:],
                    lhsT[:, :, m * P:(m + 1) * P],
                    rhs,
                    start=(kk == 0),
                    stop=(kk == KP - 1),
                    perf_mode=DR,
                )

        def epilogue(nb):
            for m in range(MT):
                if nb < 2:
                    # threshold into the paired tile; store after nb1
                    nc.vector.tensor_scalar(
                        o2[m][:, nb * NB_W:(nb + 1) * NB_W], accs[m][:],
                        0.0, None, ALU.is_le)
                    if nb == 1:
                        eng = nc.gpsimd if m % 2 == 0 else rings[(m // 2) % 2]
                        eng.dma_start(out[m * P:(m + 1) * P, 0:2 * NB_W],
                                      o2[m][:])
                else:
                    o = sb.tile([P, NB_W], F32, tag="o", bufs=4,
                                name=f"o{nb}_{m}")
                    nc.vector.tensor_scalar(o[:], accs[m][:], 0.0, None,
                                            ALU.is_le)
                    eng = nc.gpsimd if m % 2 == 0 else rings[(nb + m // 2) % 2]
                    eng.dma_start(
                        out[m * P:(m + 1) * P, nb * NB_W:(nb + 1) * NB_W],
                        o[:])

        # n-block 0: stream x + W k-pair-wise so chains ride the DMA
        for kk in range(KP):
            load_x_pair(kk, rings[kk % 2])
            load_w_pair(0, kk, rings[(kk + 1) % 2])
            mm_step(0, kk)
        epilogue(0)

        # n-blocks 1..3: W-only streams, chains consume on arrival
        for nb in range(1, NB):
            for kk in range(KP):
                load_w_pair(nb, kk, rings[kk % 2])
                mm_step(nb, kk)
            epilogue(nb)


_NC_CACHE = {}


def _get_nc():
    if "nc" not in _NC_CACHE:
        nc = bacc.Bacc("TRN2", target_bir_lowering=False, debug=False,
                       num_devices=N_CORES)
        xp = nc.dram_tensor("xp", [KP, P, 2 * M_LOCAL], F32,
                            kind="ExternalInput")
        wp = nc.dram_tensor("wp", [KP * NB, P, 2 * NB_W], F32,
                            kind="ExternalInput")
        out = nc.dram_tensor("out", [M_LOCAL, N_RULES], F32,
                             kind="ExternalOutput")
        with tile.TileContext(nc) as tc:
            _body(tc, out.ap(), xp.ap(), wp.ap())
        nc.compile()
        _NC_CACHE["nc"] = nc
    return _NC_CACHE["nc"]


def _permute_w(W: np.ndarray) -> np.ndarray:
    # [IN_DIM, N_RULES] -> [KP*NB, P, 2*NB_W]: for each k-pair kk and
    # n-block nb, row p holds [W[2kk*128+p, nb-block], W[(2kk+1)*128+p, ...]]
    w5 = W.reshape(KP, 2, P, NB, NB_W)          # [kk, j, p, nb, n]
    return np.ascontiguousarray(
        w5.transpose(0, 3, 2, 1, 4).reshape(KP * NB, P, 2 * NB_W))


def _permute_x(x_shard: np.ndarray) -> np.ndarray:
    # [M_LOCAL, IN_DIM] -> [KP, P, 2*M_LOCAL]: row p of slab kk holds
    # [x[:, 2kk*128+p].T, x[:, (2kk+1)*128+p].T]
    x4 = x_shard.T.reshape(KP, 2, P, M_LOCAL)   # [kk, j, p, m]
    return np.ascontiguousarray(x4.transpose(0, 2, 1, 3).reshape(
        KP, P, 2 * M_LOCAL))


def kernel(x: np.ndarray, W: np.ndarray, **run_kwargs) -> np.ndarray:
    assert x.shape == (BATCH, IN_DIM) and W.shape == (IN_DIM, N_RULES)
    x = np.ascontiguousarray(x, dtype=np.float32)
    W = np.ascontiguousarray(W, dtype=np.float32)
    nc = _get_nc()
    wp = _permute_w(W)
    in_maps = []
    for c in range(N_CORES):
        in_maps.append({"xp": _permute_x(x[c * M_LOCAL:(c + 1) * M_LOCAL, :]),
                        "wp": wp})
    res = run_bass_kernel_spmd(nc, in_maps, core_ids=list(range(N_CORES)),
                               **run_kwargs)
    out = np.concatenate([res.results[c]["out"] for c in range(N_CORES)],
                         axis=0)
    if run_kwargs:
        kernel.last_results = res
    return out


# revision 8
# speedup vs baseline: 1.0506x; 1.0506x over previous
"""Trainium2 kernel for a fuzzy-logic ConjunctionLayer forward pass.

Computes  out = 1[ (1 - x) @ 1[W > 0.5] <= 0 ]  for
x: [8192, 4096] f32, W: [4096, 2048] f32 -> out: [8192, 2048] f32.

Sharding: data-parallel over the batch dim across 8 NeuronCores
(x shard [1024, 4096] per core, W replicated), outputs concatenated.

Math: with x in [0, 1], every term (1-x)*Wb is >= 0, so
  res[m,n] <= 0  <=>  res[m,n] == 0  <=>  no k has (x[m,k] < 1 AND W[k,n] > .5).
The output depends only on the support pattern, so both operands are
binarized on device:
  s  = 1[x < 1],  Wb = 1[W > .5]   (both {0,1}, exact in fp8e4)
  acc = s^T.T @ Wb                 (f32 PSUM accumulation - exact integers)
  out = 1[acc <= 0]
fp8 enables the PE DoubleRow perf mode (2 fp8 weights per cell -> 2x
MACs/cycle, contraction 256 per matmul).

The kernel is DMA-bound (56 MB/core), so the schedule maximizes HBM
bandwidth: every transfer is >= 0.5 MB with >= 4 KB contiguous
per-partition rows (the host pre-permutes x and W into k-pair-major
layouts to make that possible), transfers alternate across the two HWDGE
rings (Sync + Scalar issue queues), and the 8 batch-chunk accumulation
chains (one PSUM bank each) consume every k-pair the moment it lands.
W streams n-block-major so each 8 MB block is reused by all 8 chains
while the next block prefetches; output stores are split between the
GPSIMD SWDGE queue and the rings.
"""

import numpy as np

import concourse.bass as bass
import concourse.mybir as mybir
import concourse.tile as tile
from concourse import bacc
from concourse.bass_utils import run_bass_kernel_spmd

BATCH, IN_DIM, N_RULES = 8192, 4096, 2048
N_CORES = 8
M_LOCAL = BATCH // N_CORES  # 1024 batch rows per core

P = 128            # SBUF partitions / matmul tile edge
NB_W = 512         # n-block width (= one f32 PSUM bank)
NB = N_RULES // NB_W        # 4 n-blocks
KT = IN_DIM // P            # 32 k-tiles
KP = KT // 2                # 16 k-pairs (DoubleRow consumes 2 per matmul)
MT = M_LOCAL // P           # 8 batch chunks per core

F32 = mybir.dt.float32
FP8 = mybir.dt.float8e4
ALU = mybir.AluOpType
DR = mybir.MatmulPerfMode.DoubleRow


def _body(tc: tile.TileContext, out: bass.AP, xp: bass.AP, wp: bass.AP):
    nc = tc.nc
    rings = (nc.sync, nc.scalar)  # the two HWDGE issue queues
    with (
        tc.tile_pool(name="sb", bufs=1) as sb,
        tc.tile_pool(name="ps", bufs=1, space="PSUM") as ps,
    ):
        # Resident binarized operands (2D tiles; matmul slices them as
        # [128, 2, .] k-pair APs via rearrange).
        s2 = [sb.tile([P, 2 * M_LOCAL], FP8, tag=f"s{kk}", bufs=1,
                      name=f"s{kk}") for kk in range(KP)]
        wb2 = [[sb.tile([P, 2 * NB_W], FP8, tag=f"wb{nb}_{kk}", bufs=1,
                        name=f"wb{nb}_{kk}") for kk in range(KP)]
               for nb in range(NB)]
        def load_x_pair(kk):
            # two half-slab DMAs on opposite rings; deep bufs so the rings
            # always have queued transfers (a drained ring wastes HBM BW)
            for j in (0, 1):
                xf = sb.tile([P, M_LOCAL], F32, tag="xf", bufs=6,
                             name=f"xf{kk}_{j}")
                rings[(kk + j) % 2].dma_start(xf[:], xp[kk, :,
                                                        j * M_LOCAL:
                                                        (j + 1) * M_LOCAL])
                nc.vector.tensor_scalar(s2[kk][:, j * M_LOCAL:
                                               (j + 1) * M_LOCAL],
                                        xf[:], 1.0, None, ALU.is_lt)

        def load_w_pair(nb, kk, ring):
            wf = sb.tile([P, 2 * NB_W], F32, tag="wf", bufs=8,
                         name=f"wf{nb}_{kk}")
            ring.dma_start(wf[:], wp[kk * NB + nb])
            nc.vector.tensor_scalar(wb2[nb][kk][:], wf[:], 0.5, None,
                                    ALU.is_gt)

        accs = {}

        def mm_step(nb, kk):
            """All 8 batch chains consume k-pair kk of n-block nb."""
            rhs = wb2[nb][kk][:].rearrange("p (two n) -> p two n", two=2)
            lhsT = s2[kk][:].rearrange("p (two m) -> p two m", two=2)
            for m in range(MT):
                if kk == 0:
                    accs[m] = ps.tile([P, NB_W], F32, tag=f"acc{m}", bufs=1,
                                      name=f"acc{nb}_{m}")
                nc.tensor.matmul(
                    accs[m][:],
                    lhsT[:, :, m * P:(m + 1) * P],
                    rhs,
                    start=(kk == 0),
                    stop=(kk == KP - 1),
                    perf_mode=DR,
                )

        def epilogue(nb):
            for m in range(MT):
                o = sb.tile([P, NB_W], F32, tag="o", bufs=6,
                            name=f"o{nb}_{m}")
                nc.vector.tensor_scalar(o[:], accs[m][:], 0.0, None,
                                        ALU.is_le)
                eng = nc.gpsimd if m % 2 == 0 else rings[(nb + m // 2) % 2]
                eng.dma_start(
                    out[m * P:(m + 1) * P, nb * NB_W:(nb + 1) * NB_W],
                    o[:])

        # n-block 0: stream x + W k-pair-wise so chains ride the DMA
        for kk in range(KP):
            load_x_pair(kk)
            load_w_pair(0, kk, rings[kk % 2])
            mm_step(0, kk)
        epilogue(0)

        # n-blocks 1..3: W-only streams, chains consume on arrival
        for nb in range(1, NB):
            for kk in range(KP):
                load_w_pair(nb, kk, rings[kk % 2])
                mm_step(nb, kk)
            epilogue(nb)


_NC_CACHE = {}


def _get_nc():
    if "nc" not in _NC_CACHE:
        nc = bacc.Bacc("TRN2", target_bir_lowering=False, debug=False,
                       num_devices=N_CORES)
        xp = nc.dram_tensor("xp", [KP, P, 2 * M_LOCAL], F32,
                            kind="ExternalInput")
        wp = nc.dram_tensor("wp", [KP * NB, P, 2 * NB_W], F32,
                            kind="ExternalInput")
        out = nc.dram_tensor("out", [M_LOCAL, N_RULES], F32,
                             kind="ExternalOutput")
        with tile.TileContext(nc) as tc:
            _body(tc, out.ap(), xp.ap(), wp.ap())
        nc.compile()
        _NC_CACHE["nc"] = nc
    return _NC_CACHE["nc"]


def _permute_w(W: np.ndarray) -> np.ndarray:
    # [IN_DIM, N_RULES] -> [KP*NB, P, 2*NB_W]: for each k-pair kk and
    # n-block nb, row p holds [W[2kk*128+p, nb-block], W[(2kk+1)*128+p, ...]]
    w5 = W.reshape(KP, 2, P, NB, NB_W)          # [kk, j, p, nb, n]
    return np.ascontiguousarray(
        w5.transpose(0, 3, 2, 1, 4).reshape(KP * NB, P, 2 * NB_W))


def _permute_x(x_shard: np.ndarray) -> np.ndarray:
    # [M_LOCAL, IN_DIM] -> [KP, P, 2*M_LOCAL]: row p of slab kk holds
    # [x[:, 2kk*128+p].T, x[:, (2kk+1)*128+p].T]
    x4 = x_shard.T.reshape(KP, 2, P, M_LOCAL)   # [kk, j, p, m]
    return np.ascontiguousarray(x4.transpose(0, 2, 1, 3).reshape(
        KP, P, 2 * M_LOCAL))


def kernel(x: np.ndarray, W: np.ndarray, **run_kwargs) -> np.ndarray:
    assert x.shape == (BATCH, IN_DIM) and W.shape == (IN_DIM, N_RULES)
    x = np.ascontiguousarray(x, dtype=np.float32)
    W = np.ascontiguousarray(W, dtype=np.float32)
    nc = _get_nc()
    wp = _permute_w(W)
    in_maps = []
    for c in range(N_CORES):
        in_maps.append({"xp": _permute_x(x[c * M_LOCAL:(c + 1) * M_LOCAL, :]),
                        "wp": wp})
    res = run_bass_kernel_spmd(nc, in_maps, core_ids=list(range(N_CORES)),
                               **run_kwargs)
    out = np.concatenate([res.results[c]["out"] for c in range(N_CORES)],
                         axis=0)
    if run_kwargs:
        kernel.last_results = res
    return out


# revision 13
# speedup vs baseline: 1.1112x; 1.0576x over previous
"""Trainium2 kernel for a fuzzy-logic ConjunctionLayer forward pass.

Computes  out = 1[ (1 - x) @ 1[W > 0.5] <= 0 ]  for
x: [8192, 4096] f32, W: [4096, 2048] f32 -> out: [8192, 2048] f32.

Sharding: data-parallel over the batch dim across 8 NeuronCores
(x shard [1024, 4096] per core, W replicated), outputs concatenated.

Math: with x in [0, 1], every term (1-x)*Wb is >= 0, so
  res[m,n] <= 0  <=>  res[m,n] == 0  <=>  no k has (x[m,k] < 1 AND W[k,n] > .5).
The output depends only on the support pattern, so both operands are
binarized on device:
  s  = 1[x < 1],  Wb = 1[W > .5]   (both {0,1}, exact in fp8e4)
  acc = s^T.T @ Wb                 (f32 PSUM accumulation - exact integers)
  out = 1[acc <= 0]
fp8 enables the PE DoubleRow perf mode (2 fp8 weights per cell -> 2x
MACs/cycle, contraction 256 per matmul).

The kernel is DMA-bound (56 MB/core), so the schedule maximizes HBM
bandwidth: every transfer is >= 0.5 MB with >= 4 KB contiguous
per-partition rows (the host pre-permutes x and W into k-pair-major
layouts to make that possible), transfers alternate across the two HWDGE
rings (Sync + Scalar issue queues), and the 8 batch-chunk accumulation
chains (one PSUM bank each) consume every k-pair the moment it lands.
W streams n-block-major so each 8 MB block is reused by all 8 chains
while the next block prefetches; output stores are split between the
GPSIMD SWDGE queue and the rings.
"""

import numpy as np

import concourse.bass as bass
import concourse.mybir as mybir
import concourse.tile as tile
from concourse import bacc
from concourse.bass_utils import run_bass_kernel_spmd

BATCH, IN_DIM, N_RULES = 8192, 4096, 2048
N_CORES = 8
M_LOCAL = BATCH // N_CORES  # 1024 batch rows per core

P = 128            # SBUF partitions / matmul tile edge
NB_W = 512         # n-block width (= one f32 PSUM bank)
NB = N_RULES // NB_W        # 4 n-blocks
KT = IN_DIM // P            # 32 k-tiles
KP = KT // 2                # 16 k-pairs (DoubleRow consumes 2 per matmul)
MT = M_LOCAL // P           # 8 batch chunks per core

F32 = mybir.dt.float32
BF16 = mybir.dt.bfloat16
FP8 = mybir.dt.float8e4
ALU = mybir.AluOpType
DR = mybir.MatmulPerfMode.DoubleRow


def _body(tc: tile.TileContext, out: bass.AP, xp: bass.AP, wp: bass.AP):
    nc = tc.nc
    rings = (nc.sync, nc.scalar)  # the two HWDGE issue queues
    with (
        tc.tile_pool(name="sb", bufs=1) as sb,
        tc.tile_pool(name="ps", bufs=1, space="PSUM") as ps,
    ):
        # Resident binarized operands (2D tiles; matmul slices them as
        # [128, 2, .] k-pair APs via rearrange).
        s2 = [sb.tile([P, 2 * M_LOCAL], FP8, tag=f"s{kk}", bufs=1,
                      name=f"s{kk}") for kk in range(KP)]
        wb2 = [[sb.tile([P, 2 * NB_W], FP8, tag=f"wb{nb}_{kk}", bufs=1,
                        name=f"wb{nb}_{kk}") for kk in range(KP)]
               for nb in range(NB)]
        def load_x_pair(kk):
            # two half-slab DMAs on opposite rings; deep bufs so the rings
            # always have queued transfers (a drained ring wastes HBM BW)
            for j in (0, 1):
                xf = sb.tile([P, M_LOCAL], F32, tag="xf", bufs=8,
                             name=f"xf{kk}_{j}")
                rings[(kk + j) % 2].dma_start(xf[:], xp[kk, :,
                                                        j * M_LOCAL:
                                                        (j + 1) * M_LOCAL])
                nc.vector.tensor_scalar(s2[kk][:, j * M_LOCAL:
                                               (j + 1) * M_LOCAL],
                                        xf[:], 1.0, None, ALU.is_lt)

        def load_w_pair(nb, kk, split=False):
            wf = sb.tile([P, 2 * NB_W], F32, tag="wf", bufs=8,
                         name=f"wf{nb}_{kk}")
            if split:
                # halve across both rings for exact per-k-pair balance
                for j in (0, 1):
                    rings[(kk + j) % 2].dma_start(
                        wf[:, j * NB_W:(j + 1) * NB_W],
                        wp[kk * NB + nb, :, j * NB_W:(j + 1) * NB_W])
            else:
                rings[kk % 2].dma_start(wf[:], wp[kk * NB + nb])
            nc.vector.tensor_scalar(wb2[nb][kk][:], wf[:], 0.5, None,
                                    ALU.is_gt)

        accs = {}

        def mm_step(nb, kk):
            """All 8 batch chains consume k-pair kk of n-block nb."""
            rhs = wb2[nb][kk][:].rearrange("p (two n) -> p two n", two=2)
            lhsT = s2[kk][:].rearrange("p (two m) -> p two m", two=2)
            for m in range(MT):
                if kk == 0:
                    accs[m] = ps.tile([P, NB_W], F32, tag=f"acc{m}", bufs=1,
                                      name=f"acc{nb}_{m}")
                nc.tensor.matmul(
                    accs[m][:],
                    lhsT[:, :, m * P:(m + 1) * P],
                    rhs,
                    start=(kk == 0),
                    stop=(kk == KP - 1),
                    perf_mode=DR,
                )

        def epilogue(nb):
            # bf16 stores: {0,1} is exact in bf16 and halves write traffic;
            # the host widens back to f32.
            for m in range(MT):
                o = sb.tile([P, NB_W], BF16, tag="o", bufs=6,
                            name=f"o{nb}_{m}")
                nc.vector.tensor_scalar(o[:], accs[m][:], 0.0, None,
                                        ALU.is_le)
                eng = nc.gpsimd if m % 2 == 0 else rings[(nb + m // 2) % 2]
                eng.dma_start(
                    out[m * P:(m + 1) * P, nb * NB_W:(nb + 1) * NB_W],
                    o[:])

        # n-block 0: stream x + W k-pair-wise so chains ride the DMA
        for kk in range(KP):
            load_x_pair(kk)
            load_w_pair(0, kk, split=True)
            mm_step(0, kk)
        epilogue(0)

        # n-blocks 1..3: W-only streams, chains consume on arrival
        for nb in range(1, NB):
            for kk in range(KP):
                load_w_pair(nb, kk)
                mm_step(nb, kk)
            epilogue(nb)


_NC_CACHE = {}


def _get_nc():
    if "nc" not in _NC_CACHE:
        nc = bacc.Bacc("TRN2", target_bir_lowering=False, debug=False,
                       num_devices=N_CORES)
        xp = nc.dram_tensor("xp", [KP, P, 2 * M_LOCAL], F32,
                            kind="ExternalInput")
        wp = nc.dram_tensor("wp", [KP * NB, P, 2 * NB_W], F32,
                            kind="ExternalInput")
        out = nc.dram_tensor("out", [M_LOCAL, N_RULES], BF16,
                             kind="ExternalOutput")
        with tile.TileContext(nc) as tc:
            _body(tc, out.ap(), xp.ap(), wp.ap())
        nc.compile()
        _NC_CACHE["nc"] = nc
    return _NC_CACHE["nc"]


def _permute_w(W: np.ndarray) -> np.ndarray:
    # [IN_DIM, N_RULES] -> [KP*NB, P, 2*NB_W]: for each k-pair kk and
    # n-block nb, row p holds [W[2kk*128+p, nb-block], W[(2kk+1)*128+p, ...]]
    w5 = W.reshape(KP, 2, P, NB, NB_W)          # [kk, j, p, nb, n]
    return np.ascontiguousarray(
        w5.transpose(0, 3, 2, 1, 4).reshape(KP * NB, P, 2 * NB_W))


def _permute_x(x_shard: np.ndarray) -> np.ndarray:
    # [M_LOCAL, IN_DIM] -> [KP, P, 2*M_LOCAL]: row p of slab kk holds
    # [x[:, 2kk*128+p].T, x[:, (2kk+1)*128+p].T]
    x4 = x_shard.T.reshape(KP, 2, P, M_LOCAL)   # [kk, j, p, m]
    return np.ascontiguousarray(x4.transpose(0, 2, 1, 3).reshape(
        KP, P, 2 * M_LOCAL))


def kernel(x: np.ndarray, W: np.ndarray, **run_kwargs) -> np.ndarray:
    assert x.shape == (BATCH, IN_DIM) and W.shape == (IN_DIM, N_RULES)
    x = np.ascontiguousarray(x, dtype=np.float32)
    W = np.ascontiguousarray(W, dtype=np.float32)
    nc = _get_nc()
    wp = _permute_w(W)
    in_maps = []
    for c in range(N_CORES):
        in_maps.append({"xp": _permute_x(x[c * M_LOCAL:(c + 1) * M_LOCAL, :]),
                        "wp": wp})
    res = run_bass_kernel_spmd(nc, in_maps, core_ids=list(range(N_CORES)),
                               **run_kwargs)
    out = np.concatenate([res.results[c]["out"] for c in range(N_CORES)],
                         axis=0).astype(np.float32)  # bf16 {0,1} -> f32 exact
    if run_kwargs:
        kernel.last_results = res
    return out
